# revision 11
# baseline (speedup 1.0000x reference)
"""Trainium2 Bass kernel for nn_DRUCell: 8-way data-parallel DRU cell.

reference:
    xh = concat([x, h], 1)                  # [B, IN+H]
    lin = xh @ W_in + b_in                  # [B, 2H]
    learn = tanh(lin[:, :H]); f = sigmoid(lin[:, H:])
    h_new = f * h + (1 - f) * learn
    out = tanh(concat([x, h_new], 1) @ W_out + b_out)
    returns (out, h_new)

Strategy: shard batch across the 8 NeuronCores (2048 rows each), replicate
weights. On-device everything lives feature-major ([feature, batch]) so the
TensorE contraction (over features) maps to partitions with no on-device
transposes; the host pre-transposes the shards (free relative to HW time) and
transposes the outputs back.

The pipeline runs in fp16 (fp32 PSUM accumulation, fp32 biases): PE streams
fp16 at the same 1 col/cycle as bf16 but with 3 more mantissa bits, the DVE
h_new chain gets 2x 16-bit throughput, there are no cast ops at all, and the
h-load plus both output stores halve their HBM bytes.

The forget gate's matmul runs in fp8e4m3 with perf_mode=DoubleRow (the PE
packs 2 fp8 weights/cell, virtualizing the array to 256 contraction rows):
128 fp16 matmuls become 64 DoubleRow matmuls at ~1.44x throughput. Only the
forget pre-activation tolerates fp8 noise -- it passes through sigmoid
(derivative <= 1/4) and multiplies h, adding ~1% to h_new / ~0.6% to out
(measured; gate is 2%). learn/mm2 in fp8 would breach the gate.

Schedule notes (from NTFF profiles):
- All loads ride one HWDGE ring (SP), criticality-ordered; stores ride the
  ACT ring so the store stream never queues behind the load stream.
- A short stream of dummy matmuls runs during the load phase to warm the PE
  HAM clock gate, so the real matmuls start at 2.4 GHz.
- A dummy SIGMOID runs before any real activation so walrus's first
  ACT_TABLE_LOAD fetches the sigmoid set (which also contains tanh).
- Tile 0's learn matmuls run k-outer so every arriving W chunk unlocks four
  matmuls (dense PE during the load trickle); its fp8 forget block runs
  after, by which time the fp8 operands have arrived. Later tiles run
  per-chunk (learn 8 MMs, forget 4 DR MMs) so the h_new DVE chain for chunk
  c overlaps the remaining chunks.
- mm2 runs k-outer into one 4-bank PSUM tile; h_new chunks are stored
  per-chunk as they finish. The last tile's mm2 groups stop staggered so the
  tail activations/stores overlap the final matmuls.
- Stores stay on HWDGE; the GpSimd SWDGE path hard-hangs the device with
  these 3D access patterns.
"""

import numpy as np
import ml_dtypes
from contextlib import ExitStack

import concourse.bass as bass
import concourse.mybir as mybir
import concourse.tile as tile
from concourse import bacc
from concourse.bass_utils import run_bass_kernel_spmd

B, IN, H = 16384, 512, 512
NCORES = 8
BL = B // NCORES  # batch rows per core
P = 128
NB = 512          # batch columns per device tile
NT = BL // NB
KIN = IN // P     # x feature chunks
KH = H // P       # h feature chunks
K1 = KIN + KH     # contraction chunks for both matmuls
KP = K1 // 2      # DoubleRow chunk-pairs for the fp8 forget matmul
MO1 = 2 * H // P  # mm1 output chunks (learn 0..KH-1, forget KH..)
MO2 = H // P      # mm2 output chunks
N_WARMUP = 8      # dummy matmuls to warm the PE HAM gate (~3.4us cold)

MM_MODE = "f16f8"  # "f16f8" | "f16"

_nc_cache = {}


def _build(mm_mode):
    f32 = mybir.dt.float32
    f16 = mybir.dt.float16
    f8 = mybir.dt.float8e4
    use_f8 = mm_mode == "f16f8"
    DR = mybir.MatmulPerfMode.DoubleRow

    nc = bacc.Bacc("TRN2", target_bir_lowering=False, debug=False, num_devices=NCORES)

    xT_d = nc.dram_tensor("xT", [IN, BL], f16, kind="ExternalInput")
    hT_d = nc.dram_tensor("hT", [H, BL], f16, kind="ExternalInput")
    # learn half of W_in in fp16; forget half in fp8 (DoubleRow) if enabled
    w_in_d = nc.dram_tensor(
        "w_in", [IN + H, H if use_f8 else 2 * H], f16, kind="ExternalInput"
    )
    w_out_d = nc.dram_tensor("w_out", [IN + H, H], f16, kind="ExternalInput")
    b_in_d = nc.dram_tensor("b_in", [P, MO1], f32, kind="ExternalInput")
    b_out_d = nc.dram_tensor("b_out", [P, MO2], f32, kind="ExternalInput")
    if use_f8:
        x8T_d = nc.dram_tensor("x8T", [IN, BL], f8, kind="ExternalInput")
        h8T_d = nc.dram_tensor("h8T", [H, BL], f8, kind="ExternalInput")
        w8f_d = nc.dram_tensor("w8f", [IN + H, H], f8, kind="ExternalInput")
    h_newT_d = nc.dram_tensor("h_newT", [H, BL], f16, kind="ExternalOutput")
    outT_d = nc.dram_tensor("outT", [H, BL], f16, kind="ExternalOutput")

    AFT = mybir.ActivationFunctionType
    WID1 = H if use_f8 else 2 * H  # free width of a w_in chunk tile

    # feature-major DRAM views: row (c*128 + p) <-> (partition p, chunk c)
    x_dram = xT_d.ap().rearrange("(c p) n -> p c n", p=P)
    h_dram = hT_d.ap().rearrange("(c p) n -> p c n", p=P)
    w_in_dram = w_in_d.ap().rearrange("(k p) m -> p k m", p=P)
    hn_dram = h_newT_d.ap().rearrange("(c p) n -> p c n", p=P)
    out_dram = outT_d.ap().rearrange("(c p) n -> p c n", p=P)
    w_out_dram = w_out_d.ap().rearrange("(k p) m -> p k m", p=P)
    if use_f8:
        x8_dram = x8T_d.ap().rearrange("(c p) n -> p c n", p=P)
        h8_dram = h8T_d.ap().rearrange("(c p) n -> p c n", p=P)
        # DoubleRow stationary layout: [partition, kp, ko, m]
        w8f_dram = w8f_d.ap().rearrange("(kp ko p) m -> p kp ko m", p=P, ko=2)

    with tile.TileContext(nc) as tc, ExitStack() as ctx:
        cpool = ctx.enter_context(tc.tile_pool(name="consts", bufs=1))
        work = ctx.enter_context(tc.tile_pool(name="work", bufs=2))
        tmp_pool = ctx.enter_context(tc.tile_pool(name="tmp", bufs=4))
        psum1 = ctx.enter_context(tc.tile_pool(name="psum1", bufs=4, space="PSUM"))
        psum2 = ctx.enter_context(tc.tile_pool(name="psum2", bufs=1, space="PSUM"))

        x_sb = [cpool.tile([P, KIN * NB], f16, name=f"x_sb_{j}") for j in range(NT)]
        h_sb = [cpool.tile([P, KH * NB], f16, name=f"h_sb_{j}") for j in range(NT)]
        if use_f8:
            x8_sb = [
                cpool.tile([P, KIN * NB], f8, name=f"x8_sb_{j}") for j in range(NT)
            ]
            h8_sb = [
                cpool.tile([P, KH * NB], f8, name=f"h8_sb_{j}") for j in range(NT)
            ]
            w8_sb = [cpool.tile([P, 2 * H], f8, name=f"w8_{kp}") for kp in range(KP)]

        def load_x(eng, j, lo, hi):
            bs = bass.ts(j, NB)
            xv = x_sb[j][:].rearrange("p (k n) -> p k n", k=KIN)
            eng.dma_start(xv[:, lo:hi, :], x_dram[:, lo:hi, bs])

        def load_h(eng, j, lo, hi):
            bs = bass.ts(j, NB)
            hv = h_sb[j][:].rearrange("p (c n) -> p c n", c=KH)
            eng.dma_start(hv[:, lo:hi, :], h_dram[:, lo:hi, bs])

        def load_f8(eng, j):
            bs = bass.ts(j, NB)
            xv = x8_sb[j][:].rearrange("p (k n) -> p k n", k=KIN)
            eng.dma_start(xv[:], x8_dram[:, :, bs])
            hv = h8_sb[j][:].rearrange("p (c n) -> p c n", c=KH)
            eng.dma_start(hv[:], h8_dram[:, :, bs])

        def f8_rhs(j, kp):
            # chunk pair (2kp, 2kp+1) of the combined [x; h] contraction
            if kp < KIN // 2:
                src, k = x8_sb[j], KIN
            else:
                src, kp2 = h8_sb[j], kp - KIN // 2
                kp = kp2
                k = KH
            return src[:].rearrange("p (k n) -> p k n", k=k)[
                :, 2 * kp:2 * kp + 2, :
            ]

        def f8_lhsT(kp, c):
            return w8_sb[kp][:].rearrange("p (ko m) -> p ko m", ko=2)[
                :, :, c * P:(c + 1) * P
            ]

        # Loads split across the two HWDGE rings by need-time (transfers are
        # FIFO per ring): the SP ring carries the fp16 mm1 path (W_in trickle
        # interleaved with tile 0's x chunks, then h, then later tiles); the
        # ACT ring -- idle until the first h_new store at ~19us -- carries
        # the fp8 operands and W_out so tile 1's data isn't queued behind
        # them on SP.
        w_in_sb = [cpool.tile([P, WID1], f16, name=f"w_in_{k}") for k in range(K1)]
        w_out_sb = [cpool.tile([P, H], f16, name=f"w_out_{k}") for k in range(K1)]
        b_in_sb = cpool.tile([P, MO1], f32, name="b_in_sb")
        nc.scalar.dma_start(b_in_sb[:], b_in_d[:])
        b_out_sb = cpool.tile([P, MO2], f32, name="b_out_sb")
        nc.scalar.dma_start(b_out_sb[:], b_out_d[:])

        # SP ring, strict need-order. With fp8: tile 0's forget operands
        # first (its DR matmuls open the real-work stream), W_in chunk k just
        # ahead of its x chunk, then h0, W_out interleaved with tile 1, then
        # the deep tiles.
        if use_f8:
            # kp=0 needs only w8[0] + x8_0: ready ~7.5us so the forget DR
            # stream starts right as the warmups drain
            def load_w8(kp):
                nc.sync.dma_start(
                    w8_sb[kp][:].rearrange("p (ko m) -> p ko m", ko=2),
                    w8f_dram[:, kp, :, :],
                )
            load_w8(0)
            bs0 = bass.ts(0, NB)
            x8v = x8_sb[0][:].rearrange("p (k n) -> p k n", k=KIN)
            nc.sync.dma_start(x8v[:], x8_dram[:, :, bs0])
            load_w8(1)
            h8v = h8_sb[0][:].rearrange("p (c n) -> p c n", c=KH)
            nc.sync.dma_start(h8v[:], h8_dram[:, :, bs0])
            load_w8(2)
            load_w8(3)
        nc.sync.dma_start(w_in_sb[0][:], w_in_dram[:, 0, :])
        load_x(nc.sync, 0, 0, 1)
        load_x(nc.sync, 0, 1, 2)
        nc.sync.dma_start(w_in_sb[1][:], w_in_dram[:, 1, :])
        load_x(nc.sync, 0, 2, 3)
        nc.sync.dma_start(w_in_sb[2][:], w_in_dram[:, 2, :])
        load_x(nc.sync, 0, 3, KIN)
        nc.sync.dma_start(w_in_sb[3][:], w_in_dram[:, 3, :])
        load_h(nc.sync, 0, 0, KH)
        for k in range(4, K1):
            nc.sync.dma_start(w_in_sb[k][:], w_in_dram[:, k, :])
        for k in range(0, 4):
            nc.sync.dma_start(w_out_sb[k][:], w_out_dram[:, k, :])
        load_x(nc.sync, 1, 0, KIN)
        for k in range(4, 6):
            nc.sync.dma_start(w_out_sb[k][:], w_out_dram[:, k, :])
        load_h(nc.sync, 1, 0, KH)
        for k in range(6, K1):
            nc.sync.dma_start(w_out_sb[k][:], w_out_dram[:, k, :])
        if use_f8:
            load_f8(nc.sync, 1)
        for j in range(2, NT):
            load_x(nc.sync, j, 0, KIN)
            load_h(nc.sync, j, 0, KH)
            if use_f8:
                load_f8(nc.sync, j)

        # ---- PE warm-up: dummy matmuls on a memset tile while loads run ----
        warm_src = cpool.tile([P, NB], f16, name="warm_src")
        nc.gpsimd.memset(warm_src[:], 0.0)
        # Table preload: the first activation in queue order is a SIGMOID, so
        # the ACT_TABLE_LOAD fetching the sigmoid set happens off the
        # critical path.
        warm_act = cpool.tile([P, 16], f16, name="warm_act")
        nc.scalar.activation(warm_act[:], warm_src[:, 0:16], AFT.Sigmoid)
        for w in range(N_WARMUP):
            wps = psum1.tile([P, NB], f32, name="warm_ps", tag="ps1")
            nc.tensor.matmul(
                wps[:], warm_src[:, 0:P], warm_src[:], start=True, stop=True
            )

        def mm1_learn(ps, k, c, j):
            rhs = (
                x_sb[j][:, bass.ts(k, NB)]
                if k < KIN
                else h_sb[j][:, bass.ts(k - KIN, NB)]
            )
            nc.tensor.matmul(
                ps,
                w_in_sb[k][:, c * P:(c + 1) * P],
                rhs,
                start=(k == 0),
                stop=(k == K1 - 1),
            )

        def mm1_forget_f16(ps, k, c, j):
            rhs = (
                x_sb[j][:, bass.ts(k, NB)]
                if k < KIN
                else h_sb[j][:, bass.ts(k - KIN, NB)]
            )
            nc.tensor.matmul(
                ps,
                w_in_sb[k][:, (c + KH) * P:(c + KH + 1) * P],
                rhs,
                start=(k == 0),
                stop=(k == K1 - 1),
            )

        def mm1_forget_f8(ps, kp, c, j):
            nc.tensor.matmul(
                ps,
                f8_lhsT(kp, c),
                f8_rhs(j, kp),
                start=(kp == 0),
                stop=(kp == KP - 1),
                perf_mode=DR,
            )

        for j in range(NT):
            bs = bass.ts(j, NB)

            learn = work.tile([P, KH * NB], f16, name="learn", tag="learn")
            forget = work.tile([P, KH * NB], f16, name="forget", tag="forget")
            hn = work.tile([P, KH * NB], f16, name="hn", tag="hn")

            def elemwise(c):
                cs = bass.ts(c, NB)
                t = tmp_pool.tile([P, NB], f16, name="t", tag="t")
                t2 = tmp_pool.tile([P, NB], f16, name="t2", tag="t2")
                nc.vector.tensor_sub(t[:], h_sb[j][:, cs], learn[:, cs])
                nc.vector.tensor_mul(t2[:], t[:], forget[:, cs])
                nc.vector.tensor_add(hn[:, cs], t2[:], learn[:, cs])
                # store this h_new chunk right away (ACT ring)
                nc.scalar.dma_start(
                    hn_dram[:, c:c + 1, bs],
                    hn[:, cs].rearrange("p (c n) -> p c n", c=1),
                )

            if j == 0:
                # fp8 forget first: its operands are the first loads, so its
                # DR matmuls open the real-work stream while W_in trickles
                # in; learn then runs k-outer across four psum1 banks (each
                # arriving W_in chunk unlocks 4 matmuls). Forget sigmoids
                # are emitted before the tanh/elemwise chain since their
                # PSUM is ready first.
                ps_ls = [
                    psum1.tile([P, NB], f32, name="ps_l", tag="ps1")
                    for _ in range(KH)
                ]
                ps2w = psum2.tile([P, MO2 * NB], f32, name="ps2w", tag="ps2")
                if use_f8:
                    for c in range(KH):
                        for kp in range(KP):
                            mm1_forget_f8(ps2w[:, bass.ts(c, NB)], kp, c, j)
                for k in range(K1):
                    for c in range(KH):
                        mm1_learn(ps_ls[c][:], k, c, j)
                        if not use_f8:
                            mm1_forget_f16(ps2w[:, bass.ts(c, NB)], k, c, j)
                for c in range(KH):
                    cs = bass.ts(c, NB)
                    nc.scalar.activation(
                        forget[:, cs], ps2w[:, cs], AFT.Sigmoid,
                        bias=b_in_sb[:, c + KH:c + KH + 1],
                    )
                for c in range(KH):
                    cs = bass.ts(c, NB)
                    nc.scalar.activation(
                        learn[:, cs], ps_ls[c][:], AFT.Tanh,
                        bias=b_in_sb[:, c:c + 1],
                    )
                    elemwise(c)
            else:
                for c in range(KH):
                    ps_l = psum1.tile([P, NB], f32, name="ps_l", tag="ps1")
                    ps_f = psum1.tile([P, NB], f32, name="ps_f", tag="ps1")
                    if use_f8:
                        for k in range(K1):
                            mm1_learn(ps_l[:], k, c, j)
                        for kp in range(KP):
                            mm1_forget_f8(ps_f[:], kp, c, j)
                    else:
                        for k in range(K1):
                            mm1_learn(ps_l[:], k, c, j)
                            mm1_forget_f16(ps_f[:], k, c, j)
                    cs = bass.ts(c, NB)
                    nc.scalar.activation(
                        learn[:, cs], ps_l[:], AFT.Tanh, bias=b_in_sb[:, c:c + 1]
                    )
                    nc.scalar.activation(
                        forget[:, cs], ps_f[:], AFT.Sigmoid,
                        bias=b_in_sb[:, c + KH:c + KH + 1],
                    )
                    elemwise(c)

            # mm2 k-outer into one 4-bank PSUM tile: the x-part (k<KIN)
            # streams while the last h_new chunks are still being produced;
            # hn chunk c is only needed at stage k = KIN + c.
            if j < NT - 1:
                ps2 = psum2.tile([P, MO2 * NB], f32, name="ps2", tag="ps2")
                pss = [ps2[:, bass.ts(mo, NB)] for mo in range(MO2)]
                for k in range(K1):
                    rhs = (
                        x_sb[j][:, bass.ts(k, NB)]
                        if k < KIN
                        else hn[:, bass.ts(k - KIN, NB)]
                    )
                    for mo in range(MO2):
                        nc.tensor.matmul(
                            pss[mo],
                            w_out_sb[k][:, mo * P:(mo + 1) * P],
                            rhs,
                            start=(k == 0),
                            stop=(k == K1 - 1),
                        )
            else:
                # last tile: mm1's psum1 slots are free by now and have
                # bank-granular deps, so the four groups stop staggered
                # (x-part k-outer, h-part mo-outer) and the tail
                # activations/stores overlap the final matmuls
                pss = [
                    psum1.tile([P, NB], f32, name="ps2s", tag="ps1")
                    for _ in range(MO2)
                ]
                for k in range(KIN):
                    rhs = x_sb[j][:, bass.ts(k, NB)]
                    for mo in range(MO2):
                        nc.tensor.matmul(
                            pss[mo][:],
                            w_out_sb[k][:, mo * P:(mo + 1) * P],
                            rhs,
                            start=(k == 0),
                            stop=False,
                        )
                for mo in range(MO2):
                    for k in range(KIN, K1):
                        nc.tensor.matmul(
                            pss[mo][:],
                            w_out_sb[k][:, mo * P:(mo + 1) * P],
                            hn[:, bass.ts(k - KIN, NB)],
                            start=False,
                            stop=(k == K1 - 1),
                        )
                pss = [p[:] for p in pss]
            out_t = work.tile([P, MO2 * NB], f16, name="out_t", tag="out_t")
            for mo in range(MO2):
                nc.scalar.activation(
                    out_t[:, bass.ts(mo, NB)],
                    pss[mo],
                    AFT.Tanh,
                    bias=b_out_sb[:, mo:mo + 1],
                )
                # store each out chunk as soon as its tanh lands (ACT ring)
                nc.scalar.dma_start(
                    out_dram[:, mo:mo + 1, bs],
                    out_t[:, bass.ts(mo, NB)].rearrange("p (c n) -> p c n", c=1),
                )

    nc.compile()
    return nc


def _get_nc(mm_mode):
    if mm_mode not in _nc_cache:
        _nc_cache[mm_mode] = _build(mm_mode)
    return _nc_cache[mm_mode]


def _run(x, h, W_in, b_in, W_out, b_out, mm_mode=MM_MODE, trace=False):
    x = np.asarray(x, dtype=np.float32)
    h = np.asarray(h, dtype=np.float32)
    W_in = np.asarray(W_in, dtype=np.float32)
    b_in = np.asarray(b_in, dtype=np.float32)
    W_out = np.asarray(W_out, dtype=np.float32)
    b_out = np.asarray(b_out, dtype=np.float32)

    use_f8 = mm_mode == "f16f8"
    f8 = ml_dtypes.float8_e4m3
    w_in_m = np.ascontiguousarray(
        (W_in[:, :H] if use_f8 else W_in).astype(np.float16)
    )
    w_out_m = np.ascontiguousarray(W_out.astype(np.float16))
    b_in_m = np.ascontiguousarray(b_in.reshape(MO1, P).T)
    b_out_m = np.ascontiguousarray(b_out.reshape(MO2, P).T)
    if use_f8:
        w8f_m = np.ascontiguousarray(W_in[:, H:].astype(f8))

    in_maps = []
    for i in range(NCORES):
        sl = slice(i * BL, (i + 1) * BL)
        xT = np.ascontiguousarray(x[sl].T)
        hT = np.ascontiguousarray(h[sl].T)
        m = {
            "xT": xT.astype(np.float16),
            "hT": hT.astype(np.float16),
            "w_in": w_in_m,
            "w_out": w_out_m,
            "b_in": b_in_m,
            "b_out": b_out_m,
        }
        if use_f8:
            m["x8T"] = xT.astype(f8)
            m["h8T"] = hT.astype(f8)
            m["w8f"] = w8f_m
        in_maps.append(m)

    nc = _get_nc(mm_mode)
    res = run_bass_kernel_spmd(nc, in_maps, list(range(NCORES)), trace=trace)

    out = np.empty((B, H), dtype=np.float32)
    h_new = np.empty((B, H), dtype=np.float32)
    for i in range(NCORES):
        sl = slice(i * BL, (i + 1) * BL)
        out[sl] = res.results[i]["outT"].T.astype(np.float32)
        h_new[sl] = res.results[i]["h_newT"].T.astype(np.float32)
    return (out, h_new), res


def kernel(x, h, W_in, b_in, W_out, b_out):
    (out, h_new), _ = _run(x, h, W_in, b_in, W_out, b_out)
    return (out, h_new)
